# revision 19
# baseline (speedup 1.0000x reference)
"""Causal multi-head attention (nn_Attention_87840671138123) on 8 trn2 NeuronCores.

Problem (B=2, S=2048, D=1024, H=16 heads, E=64 head_dim), fp32:
    Q = einsum('bsd,hde->bhse', q, W_q)   (same for K, V)
    scores = Q @ K^T / sqrt(D), causal mask, softmax
    attn = probs @ V  -> [B, S, D] (head-major concat)
    out = attn @ W_o.T

Sharding: core = 4*b + quad. Each core handles batch b and a quad of 4 heads
(heads 4*quad .. 4*quad+3). It computes a partial output
    out_part = attn_quad @ W_o.T[quad rows, :]   [S, D]  (fp16)
and the host sums the 4 partials per batch (the "all-reduce" of the output
projection done host-side at gather time).

Device layout choices (per core):
 - Host pre-tiles x^T as [P, NJ, ND, SJ] so each j-tile DMA is one
   contiguous 4-8KB segment per partition (large descriptors, fast).
 - QK path runs in fp8 e4m3 with W_q/W_k pre-scaled by SW=32 (so weights
   and Q/K values sit in e4m3's normal range); the 1/SW^2 is folded into
   the exp scale. Projections use DoubleRow fp8 matmuls (2 d-chunks per
   instruction at 0.5 cyc/col = 2x bf16 throughput).
 - QT8/KT8 are stored [P(e), 2, S]: k-tile 0 holds the values, k-tile 1 is
   zeros. The scores matmul is a DoubleRow fp8 matmul on the 64-partition
   e-range of one head (contraction = 2 k-tiles x 64; the zero k-tile
   contributes nothing) -> 0.5 cyc/col, 2x the bf16 baseline.
 - V path (V proj, probs@V, W_o) stays fp16: value-path quantization error
   passes straight to the output, fp8 there would cost ~6% error.
 - V_aug blocks: h2=0 -> [V(cols 0:64) | 1.0(cols 64:128)], h2=1 ->
   [1.0(cols 0:64) | V(cols 64:128)]. The 1.0 columns make the attn matmul
   accumulate the softmax denominator into the complementary PSUM rows, so
   h2=1's normalized output lands on partitions 64..127 directly (no
   partition-shift DMA), and h2=0's denominator is already on row 64.
 - Softmax epilogue: reciprocal_approx_fast (single custom-DVE op, ~5x
   faster than InstReciprocal) straight off the PSUM denominator row, then
   gpsimd partition_broadcast to the other half, then one multiply per h2.
   No SBUF-bounce DMAs.
 - Output projection: out[s, :] = sum_g (attnT chunk).T @ W_o.T slice,
   emitted one s-tile behind the attention pipeline; fp16 partials.
"""

import ml_dtypes
import numpy as np

import concourse.bass as bass
import concourse.tile as tile
from concourse import bacc, mybir
from concourse.bass_utils import run_bass_kernel_spmd

B, S, D, H, E = 2, 2048, 1024, 16, 64
P = 128
NCORES = 8
SJ = 512            # s-tile width
NJ = S // SJ        # 4 s-tiles
ND = D // P         # 8 d-chunks
NT = S // P         # 16 t-chunks
SW = 32.0           # host-side scale on W_q/W_k (fp8 range), folded into exp
f32 = mybir.dt.float32
bf16 = mybir.dt.bfloat16
fp16 = mybir.dt.float16
fp8 = mybir.dt.float8e4
EXP = mybir.ActivationFunctionType.Exp
MULT = mybir.AluOpType.mult
DR = mybir.MatmulPerfMode.DoubleRow

FP8_PROJ = True     # fp8 DoubleRow Q/K projections (else bf16)
V_DT = fp16         # V-path dtype
XQ_DT = fp8 if FP8_PROJ else bf16

# float8_e4m3fn: bit-identical to HW e4m3 for |x| < 240 (all our values),
# and unlike ml_dtypes.float8_e4m3 it round-trips through the jax input path
_NP_OF = {bf16: ml_dtypes.bfloat16, fp16: np.float16, f32: np.float32,
          fp8: ml_dtypes.float8_e4m3fn}

_NC_CACHE = []


def _patch_ldw_opt():
    """Enable walrus LDWEIGHTS optimization (fast weight load).

    bass_utils.bir_verify_and_optimise hardcodes --enable-ldw-opt=false;
    wrap it to rewrite the flag. Verified numerically by the test harness.
    """
    from concourse import bass_utils as _bu
    if getattr(_bu, "_ldw_patched", False):
        return
    _orig_run = _bu.run_command

    def _run(argv, **kw):
        argv = ["--enable-ldw-opt=true" if a == "--enable-ldw-opt=false" else a
                for a in argv]
        return _orig_run(argv, **kw)

    _bu.run_command = _run
    _bu._ldw_patched = True


LDW_OPT = False     # walrus ldw-opt is incompatible with DoubleRow matmuls


def _build():
    if LDW_OPT:
        _patch_ldw_opt()
    nc = bacc.Bacc("TRN2", target_bir_lowering=False, debug=False)

    # host-pre-tiled inputs: x [P, NJ, ND, SJ]; weights [P, ...] contiguous
    qT_d = nc.dram_tensor("qT", [P, NJ, ND, SJ], XQ_DT, kind="ExternalInput")
    kT_d = nc.dram_tensor("kT", [P, NJ, ND, SJ], XQ_DT, kind="ExternalInput")
    vT_d = nc.dram_tensor("vT", [P, NJ, ND, SJ], V_DT, kind="ExternalInput")
    wq_d = nc.dram_tensor("wq", [P, ND, 4 * E], XQ_DT, kind="ExternalInput")
    wk_d = nc.dram_tensor("wk", [P, ND, 4 * E], XQ_DT, kind="ExternalInput")
    wv_d = nc.dram_tensor("wv", [P, ND, 4 * E], V_DT, kind="ExternalInput")
    wot_d = nc.dram_tensor("wot", [P, 2, D], V_DT, kind="ExternalInput")
    tri_d = nc.dram_tensor("tri", [P, P], V_DT, kind="ExternalInput")
    out_d = nc.dram_tensor("out", [S, D], fp16, kind="ExternalOutput")

    with tile.TileContext(nc) as tc:
        with (
            tc.tile_pool(name="pers", bufs=1) as pers,
            tc.tile_pool(name="xt", bufs=NJ) as xt_pool,
            tc.tile_pool(name="ex", bufs=3) as ex_pool,
            tc.tile_pool(name="sm", bufs=2) as sm_pool,
            tc.tile_pool(name="ot", bufs=2) as ot_pool,
            tc.tile_pool(name="pj", bufs=2, space="PSUM") as pj_pool,
            tc.tile_pool(name="sc", bufs=2, space="PSUM") as sc_pool,
            tc.tile_pool(name="at", bufs=4, space="PSUM") as at_pool,
        ):
            # ---- persistent weights / constants (first uses first) ----
            wq_sb = pers.tile([P, ND, 4 * E], XQ_DT, name="wq_sb")
            nc.sync.dma_start(wq_sb[:], wq_d.ap())
            # pre-issue all x tiles; they land well before compute needs them
            xq = [xt_pool.tile([P, ND, SJ], XQ_DT, tag="xq", name=f"xq{j}")
                  for j in range(NJ)]
            xk = [xt_pool.tile([P, ND, SJ], XQ_DT, tag="xk", name=f"xk{j}")
                  for j in range(NJ)]
            xv = [xt_pool.tile([P, ND, SJ], V_DT, tag="xv", name=f"xv{j}")
                  for j in range(NJ)]
            nc.sync.dma_start(xq[0][:], qT_d.ap()[:, 0])
            wk_sb = pers.tile([P, ND, 4 * E], XQ_DT, name="wk_sb")
            nc.sync.dma_start(wk_sb[:], wk_d.ap())
            nc.sync.dma_start(xk[0][:], kT_d.ap()[:, 0])
            wv_sb = pers.tile([P, ND, 4 * E], V_DT, name="wv_sb")
            nc.sync.dma_start(wv_sb[:], wv_d.ap())
            nc.sync.dma_start(xv[0][:], vT_d.ap()[:, 0])
            wot_sb = pers.tile([P, 2, D], V_DT, name="wot_sb")
            nc.sync.dma_start(wot_sb[:], wot_d.ap())
            tri_sb = pers.tile([P, P], V_DT, name="tri_sb")
            nc.sync.dma_start(tri_sb[:], tri_d.ap())
            for j in range(1, NJ):
                nc.sync.dma_start(xq[j][:], qT_d.ap()[:, j])
                nc.sync.dma_start(xk[j][:], kT_d.ap()[:, j])
                nc.sync.dma_start(xv[j][:], vT_d.ap()[:, j])

            # ---- persistent activations ----
            # QT8 [e, ktile, s]: ktile 0 = values, ktile 1 = zeros.
            # Rows 0..63 = head 2g, rows 64..127 = head 2g+1.
            # KT8H[g][h2]: full-width zero-padded per head — only rows
            # h2*64..h2*64+63 of ktile 0 hold K^T; everything else stays 0 so
            # the scores matmul runs full 128 partitions at tile_position
            # (0,0) (quadrant-mode 64-partition matmuls misbehave on HW) and
            # the zero rows null the other head's Q in the shared rhs.
            QT8 = [pers.tile([P, 2, S], fp8, name=f"QT{g}") for g in range(2)]
            KT8H = [[pers.tile([P, 2, S], fp8, name=f"KT{g}{h2}")
                     for h2 in range(2)] for g in range(2)]
            # V_aug blocks per (t-chunk, h2): h2=0 [V | 1.0], h2=1 [1.0 | V].
            # The 1.0 cols accumulate the softmax denominator into the
            # complementary PSUM rows (incl. rows 64 / 0 used by the epilogue)
            # and keep the PE array fully fed (M=128).
            V = [pers.tile([P, NT, 2, P], V_DT, name=f"V{g}") for g in range(2)]
            attnG = [pers.tile([P, S], V_DT, name=f"attnG{g}") for g in range(2)]
            for g in range(2):
                nc.vector.memset(QT8[g][:, 1, :], 0)
                nc.vector.memset(KT8H[g][0][:], 0)
                nc.vector.memset(KT8H[g][1][:], 0)
                nc.vector.memset(V[g][:, :, 0, E:], 1.0)
                nc.vector.memset(V[g][:, :, 1, E:], 1.0)

            # ---- fused per-s-tile pipeline: projections -> attention -> output ----
            for j in range(NJ):
                js = slice(j * SJ, (j + 1) * SJ)
                for g in range(2):
                    pq = pj_pool.tile([P, SJ], f32, tag="pj", name=f"pq{j}{g}")
                    if FP8_PROJ:
                        for c2 in range(ND // 2):
                            nc.tensor.matmul(
                                pq[:], wq_sb[:, 2 * c2:2 * c2 + 2, bass.ts(g, P)],
                                xq[j][:, 2 * c2:2 * c2 + 2, :],
                                start=(c2 == 0), stop=(c2 == ND // 2 - 1),
                                perf_mode=DR)
                    else:
                        for c in range(ND):
                            nc.tensor.matmul(
                                pq[:], wq_sb[:, c, bass.ts(g, P)], xq[j][:, c, :],
                                start=(c == 0), stop=(c == ND - 1))
                    nc.vector.tensor_copy(QT8[g][:, 0, js], pq[:])

                for g in range(2):
                    pk = pj_pool.tile([P, SJ], f32, tag="pj", name=f"pk{j}{g}")
                    if FP8_PROJ:
                        for c2 in range(ND // 2):
                            nc.tensor.matmul(
                                pk[:], wk_sb[:, 2 * c2:2 * c2 + 2, bass.ts(g, P)],
                                xk[j][:, 2 * c2:2 * c2 + 2, :],
                                start=(c2 == 0), stop=(c2 == ND // 2 - 1),
                                perf_mode=DR)
                    else:
                        for c in range(ND):
                            nc.tensor.matmul(
                                pk[:], wk_sb[:, c, bass.ts(g, P)], xk[j][:, c, :],
                                start=(c == 0), stop=(c == ND - 1))
                    nc.vector.tensor_copy(KT8H[g][0][0:E, 0, js], pk[0:E, :])
                    nc.vector.tensor_copy(KT8H[g][1][E:, 0, js], pk[E:, :])

                for u in range(SJ // P):
                    t = 4 * j + u
                    pv = pj_pool.tile([P, 2 * P], f32, tag="pj",
                                      name=f"pv{j}{u}")
                    for c in range(ND):
                        nc.tensor.matmul(
                            pv[:], xv[j][:, c, bass.ts(u, P)], wv_sb[:, c, :],
                            start=(c == 0), stop=(c == ND - 1))
                    for g in range(2):
                        nc.vector.tensor_copy(
                            V[g][:, t, 0, 0:E], pv[:, g * P:g * P + E])
                        nc.vector.tensor_copy(
                            V[g][:, t, 1, 0:E], pv[:, g * P + E:g * P + 2 * E])

                # attention for both head pairs on this s-tile
                for g in range(2):
                    nblk = 4 * j + 4
                    atp = [
                        at_pool.tile([P, SJ], f32, tag="at", name=f"at{g}{j}{h2}")
                        for h2 in range(2)
                    ]
                    for cb in range(nblk):
                        col0 = max(0, cb - 4 * j) * P
                        scps = []
                        for h2 in range(2):
                            scp = sc_pool.tile(
                                [P, SJ], f32, tag="sc", name=f"sc{g}{j}{cb}{h2}")
                            nc.tensor.matmul(
                                scp[:, col0:],
                                KT8H[g][h2][:, :, bass.ts(cb, P)],
                                QT8[g][:, :, j * SJ + col0:(j + 1) * SJ],
                                start=True, stop=True, perf_mode=DR)
                            scps.append(scp)
                        for h2 in range(2):
                            scp = scps[h2]
                            ex = ex_pool.tile(
                                [P, SJ], V_DT, tag="ex", name=f"ex{g}{j}{cb}{h2}")
                            nc.scalar.activation(
                                ex[:, col0:], scp[:, col0:], EXP,
                                scale=1.0 / (SW * SW * 32.0))
                            if cb >= 4 * j:
                                nc.vector.tensor_tensor(
                                    ex[:, col0:col0 + P], ex[:, col0:col0 + P],
                                    tri_sb[:], MULT)
                            nc.tensor.matmul(
                                atp[h2][:, col0:],
                                V[g][:, cb, h2, :],
                                ex[:, col0:],
                                start=(cb == 0), stop=(cb == nblk - 1))
                    # epilogue: normalize by softmax denominator (row E of
                    # each atp tile) — baseline-exact pattern.
                    for h2 in range(2):
                        den = sm_pool.tile([E + 1, SJ], f32, tag="den",
                                           name=f"den{g}{j}{h2}")
                        nc.vector.tensor_copy(den[E:E + 1, :],
                                              atp[h2][E:E + 1, :])
                        rec = sm_pool.tile([1, SJ], f32, tag="rec",
                                           name=f"rec{g}{j}{h2}")
                        nc.sync.dma_start(rec[:], den[E:E + 1, :])
                        nc.vector.reciprocal(rec[:], rec[:])
                        recb = sm_pool.tile([E, SJ], f32, tag="recb",
                                            name=f"recb{g}{j}{h2}")
                        nc.gpsimd.partition_broadcast(recb[:], rec[:])
                        if h2 == 0:
                            nc.vector.tensor_tensor(
                                attnG[g][0:E, js], atp[h2][0:E, :], recb[:],
                                MULT)
                        else:
                            ah = sm_pool.tile([E, SJ], V_DT, tag="ah",
                                              name=f"ah{g}{j}")
                            nc.vector.tensor_tensor(
                                ah[:], atp[h2][0:E, :], recb[:], MULT)
                            nc.sync.dma_start(attnG[g][E:, js], ah[:])

                # ---- output projection, software-pipelined one s-tile behind
                # so the softmax-normalize epilogue chain never stalls the PE
                for u in range(SJ // P) if j > 0 else []:
                    si = 4 * (j - 1) + u
                    ot = ot_pool.tile([P, D], V_DT, tag="ot", name=f"ot{si}")
                    for no in range(2):
                        po = pj_pool.tile([P, SJ], f32, tag="pj",
                                          name=f"po{si}{no}")
                        for g in range(2):
                            nc.tensor.matmul(
                                po[:], attnG[g][:, bass.ts(si, P)],
                                wot_sb[:, g, bass.ts(no, SJ)],
                                start=(g == 0), stop=(g == 1))
                        nc.vector.tensor_copy(ot[:, bass.ts(no, SJ)], po[:])
                    nc.sync.dma_start(out_d.ap()[bass.ts(si, P), :], ot[:])

            # tail: output projection for the last s-tile
            for u in range(SJ // P):
                si = 4 * (NJ - 1) + u
                ot = ot_pool.tile([P, D], V_DT, tag="ot", name=f"ott{si}")
                for no in range(2):
                    po = pj_pool.tile([P, SJ], f32, tag="pj",
                                      name=f"pot{si}{no}")
                    for g in range(2):
                        nc.tensor.matmul(
                            po[:], attnG[g][:, bass.ts(si, P)],
                            wot_sb[:, g, bass.ts(no, SJ)],
                            start=(g == 0), stop=(g == 1))
                    nc.vector.tensor_copy(ot[:, bass.ts(no, SJ)], po[:])
                nc.sync.dma_start(out_d.ap()[bass.ts(si, P), :], ot[:])

    nc.compile()
    return nc


def _get_nc():
    if not _NC_CACHE:
        _NC_CACHE.append(_build())
    return _NC_CACHE[0]


def _tile_x(xT, np_dt):
    # [D, S] -> [P, NJ, ND, SJ]: x_t[p, j, o, s] = xT[o*P + p, j*SJ + s]
    t = xT.reshape(ND, P, NJ, SJ).transpose(1, 2, 0, 3)
    return np.ascontiguousarray(t).astype(np_dt)


def _in_maps(q, k, v, W_q, W_k, W_v, W_o):
    qk_np = _NP_OF[XQ_DT]
    v_np = _NP_OF[V_DT]
    tri = (np.arange(P)[:, None] <= np.arange(P)[None, :]).astype(v_np)
    xT = {}
    for b in range(B):
        xT[b] = (
            _tile_x(np.ascontiguousarray(q[b].T), qk_np),
            _tile_x(np.ascontiguousarray(k[b].T), qk_np),
            _tile_x(np.ascontiguousarray(v[b].T), v_np),
        )

    def _tile_w(w):   # [D, 4E] -> [P, ND, 4E]
        return np.ascontiguousarray(w.reshape(ND, P, 4 * E).transpose(1, 0, 2))

    maps = []
    for core in range(NCORES):
        b, quad = divmod(core, 4)
        hs = slice(4 * quad, 4 * quad + 4)
        qT_b, kT_b, vT_b = xT[b]
        # [4, D, E] -> [D, 4, E] -> [D, 256], col l*64+e = W[4q+l, d, e]
        wq = (W_q[hs] * SW).transpose(1, 0, 2).reshape(D, 4 * E)
        wk = (W_k[hs] * SW).transpose(1, 0, 2).reshape(D, 4 * E)
        wv = W_v[hs].transpose(1, 0, 2).reshape(D, 4 * E)
        # W_o[out, in] -> W_o.T rows for this quad's 256 input dims,
        # [256, D] -> [P, 2, D]
        wot = W_o[:, 4 * quad * E:4 * quad * E + 4 * E].T
        wot = wot.reshape(2, P, D).transpose(1, 0, 2)
        maps.append({
            "qT": qT_b,
            "kT": kT_b,
            "vT": vT_b,
            "wq": _tile_w(wq).astype(qk_np),
            "wk": _tile_w(wk).astype(qk_np),
            "wv": _tile_w(wv).astype(v_np),
            "wot": np.ascontiguousarray(wot).astype(v_np),
            "tri": tri,
        })
    return maps


def kernel(q, k, v, W_q, W_k, W_v, W_o, _trace=False, _trace_kwargs=None):
    q = np.asarray(q, dtype=np.float32)
    k = np.asarray(k, dtype=np.float32)
    v = np.asarray(v, dtype=np.float32)
    W_q = np.asarray(W_q, dtype=np.float32)
    W_k = np.asarray(W_k, dtype=np.float32)
    W_v = np.asarray(W_v, dtype=np.float32)
    W_o = np.asarray(W_o, dtype=np.float32)

    nc = _get_nc()
    maps = _in_maps(q, k, v, W_q, W_k, W_v, W_o)
    kwargs = dict(_trace_kwargs or {})
    res = run_bass_kernel_spmd(
        nc, maps, core_ids=list(range(NCORES)), trace=_trace, **kwargs)
    out = np.zeros((B, S, D), dtype=np.float32)
    for core in range(NCORES):
        b = core // 4
        out[b] += res.results[core]["out"].astype(np.float32)
    if _trace:
        kernel.last_results = res
    return out
